# revision 15
# baseline (speedup 1.0000x reference)
"""GQA causal attention (B=2, S=2048, D=2048, 16 Q heads / 8 KV heads) on 8 TRN2
NeuronCores, tensor-parallel over heads: core c owns Q heads {2c, 2c+1} and KV
head c, plus the matching output-column shard of wo.

Layout strategy (everything lands in its natural matmul layout, no transposes
of activations except V):
  - host: x is transposed to xT [D, B*S] and cast to bf16; wq/wk columns are
    de-interleaved per head (RoPE pairs (2i,2i+1) -> rows (i, i+64)) so RoPE
    becomes a partition-block swap + elementwise muls.
  - stage 1: QT/KT/VT [128, B*S] = w.T @ x.T accumulated over 16 k-tiles,
    RoPE applied to QT/KT in [d, row] layout via an SBUF->SBUF partition swap
    DMA and two DVE multiplies with host-built [c;c] / [-s;s] tables.
    VT is PE-transposed to V [row, d] (needed as PV stationary operand).
  - stage 2 (per head, batch, 512-wide q-chunk): scoresT [k,q] = KT-tile.T @ QT
    (causal: only k-tiles at or below the diagonal, diagonal k-tiles start at
    column offset so nothing above the diagonal is computed; the in-block
    triangle is zeroed with a 0/1 bf16 mask multiply on the exp output),
    exp on ACT with fused 1/sqrt(dh) scale (ACT runs exp only in stage 2 so
    its activation table never reloads), softmax denominators via ones-vector
    matmuls (partition_sum trick), PV = V-tile.T @ expT accumulated in PSUM,
    normalization by broadcasting reciprocal sums through a rank-1 matmul and
    one DVE multiply.
  - per batch: AllGather of the normalized attnT [256, S] shards over the 8
    cores (overlaps the next batch's attention / wo compute).
  - stage 3: out[q, oc] = attnT_full-tile.T @ wo-shard accumulated over the
    16 feature tiles; written as [B*S, 256] f32, host concatenates shards.
"""

import numpy as np

try:
    import concourse.bass as bass
except ImportError:  # pragma: no cover - container staging path
    import sys

    sys.path.insert(0, "/opt/trn_rl_repo")
    import concourse.bass as bass

import concourse.bacc as bacc
import concourse.mybir as mybir
import concourse.tile as tile
from concourse.bass_utils import run_bass_kernel_spmd
import ml_dtypes

BF16NP = ml_dtypes.bfloat16
N_CORES = 8
B = 2
DIM = 2048
HD = 128
N_HEADS = 16
N_KV = 8
HPC = N_HEADS // N_CORES  # q heads per core (2)
OC = DIM // N_CORES  # wo output columns per core (256)
KT = DIM // 128  # contraction tiles (16)

F32 = mybir.dt.float32
BF = mybir.dt.bfloat16
AF = mybir.ActivationFunctionType

SCALE = float(1.0 / np.sqrt(HD))


def build_kernel(S=2048):
    R = B * S
    nc = bacc.Bacc(None, target_bir_lowering=False, debug=False, num_devices=N_CORES)

    xt = nc.declare_dram_parameter("xt", [DIM, R], BF, isOutput=False)
    wq = nc.declare_dram_parameter("wq", [DIM, HPC * HD], BF, isOutput=False)
    wk = nc.declare_dram_parameter("wk", [DIM, HD], BF, isOutput=False)
    wv = nc.declare_dram_parameter("wv", [DIM, HD], BF, isOutput=False)
    wo = nc.declare_dram_parameter("wo", [DIM, OC], BF, isOutput=False)
    cs = nc.declare_dram_parameter("cs", [HD, R], BF, isOutput=False)
    sn = nc.declare_dram_parameter("sn", [HD, R], BF, isOutput=False)
    bm = nc.declare_dram_parameter("bm", [128, 128], BF, isOutput=False)
    eye = nc.declare_dram_parameter("eye", [128, 128], BF, isOutput=False)
    out = nc.declare_dram_parameter("out", [R, OC], F32, isOutput=True)

    with tile.TileContext(nc) as tc:
        with (
            tc.tile_pool(name="const", bufs=1) as constp,
            tc.tile_pool(name="acts", bufs=1) as actp,
            tc.tile_pool(name="x", bufs=2) as xp,
            tc.tile_pool(name="rope", bufs=3) as ropep,
            tc.tile_pool(name="exps", bufs=11) as expp,
            tc.tile_pool(name="small", bufs=3) as smallp,
            tc.tile_pool(name="outs", bufs=4) as outp,
            tc.tile_pool(name="wos", bufs=3) as wop,
            tc.tile_pool(name="ps_a", bufs=4, space="PSUM") as ps_a,
            tc.tile_pool(name="ps_wo", bufs=1, space="PSUM") as ps_wo,
            tc.tile_pool(name="ps_pv", bufs=2, space="PSUM") as ps_pv,
            tc.tile_pool(name="ps_sum", bufs=1, space="PSUM") as ps_sum,
            tc.tile_pool(name="dram", bufs=1, space="DRAM") as dramp,
        ):
            # ---------------- constants ----------------
            # Load order matters for startup latency: the first projection
            # matmuls need wq + the first xt chunk; wo is only needed in
            # stage 3 (emitted last). Weight loads go on the scalar-engine
            # HWDGE queue so they don't serialize behind the xt stream on
            # sync.
            wq_sb = constp.tile([128, KT, HPC * HD], BF, tag="wq")
            nc.scalar.dma_start(out=wq_sb[:], in_=wq.ap().rearrange("(t p) c -> p t c", p=128))
            wk_sb = constp.tile([128, KT, HD], BF, tag="wk")
            nc.scalar.dma_start(out=wk_sb[:], in_=wk.ap().rearrange("(t p) c -> p t c", p=128))
            wv_sb = constp.tile([128, KT, HD], BF, tag="wv")
            nc.scalar.dma_start(out=wv_sb[:], in_=wv.ap().rearrange("(t p) c -> p t c", p=128))
            cs_sb = constp.tile([128, R], BF, tag="cs")
            nc.scalar.dma_start(out=cs_sb[:], in_=cs.ap())
            sn_sb = constp.tile([128, R], BF, tag="sn")
            nc.scalar.dma_start(out=sn_sb[:], in_=sn.ap())
            bm_sb = constp.tile([128, 128], BF, tag="bm")
            nc.scalar.dma_start(out=bm_sb[:], in_=bm.ap())
            eye_sb = constp.tile([128, 128], BF, tag="eye")
            nc.scalar.dma_start(out=eye_sb[:], in_=eye.ap())
            ones128 = constp.tile([128, 1], BF, tag="ones128")
            nc.vector.memset(ones128[:], 1.0)
            ones1 = constp.tile([1, 128], BF, tag="ones1")
            nc.vector.memset(ones1[:], 1.0)
            wo_sb = constp.tile([128, KT, OC], BF, tag="wo")

            # per-head-per-batch activations (resident)
            qts = [actp.tile([128, R], BF, tag=f"qt{h}", name=f"qt{h}") for h in range(HPC)]
            ktile = actp.tile([128, R], BF, tag="kt")
            vtt = actp.tile([128, R], BF, tag="vt")
            v3 = actp.tile([128, R // 128, 128], BF, tag="v3")

            nJ = S // 512
            last_j_split = False  # per-head final gather measured slower
            if nJ >= 4:
                ag_groups = [[0, 1]] + [[j] for j in range(2, nJ)]
            else:
                ag_groups = [[j] for j in range(nJ)]
            piece_of = {}
            off_of = {}
            for p, grp in enumerate(ag_groups):
                for gi, j in enumerate(grp):
                    piece_of[j] = p
                    off_of[j] = gi * 512
            ag_in = [
                [
                    dramp.tile(
                        [HPC * HD, 512 * len(grp)],
                        BF,
                        tag=f"agin{b}_{p}",
                        name=f"agin{b}_{p}",
                    )
                    for p, grp in enumerate(ag_groups)
                ]
                for b in range(B)
            ]
            agh_in = [
                [
                    dramp.tile([HD, 512], BF, tag=f"aghin{b}_{h}", name=f"aghin{b}_{h}")
                    for h in range(HPC)
                ]
                for b in range(B)
            ]
            agh_out = [
                [
                    dramp.tile(
                        [N_CORES * HD, 512],
                        BF,
                        tag=f"aghout{b}_{h}",
                        name=f"aghout{b}_{h}",
                        addr_space="Shared",
                    )
                    for h in range(HPC)
                ]
                for b in range(B)
            ]
            ag_out = [
                [
                    dramp.tile(
                        [N_CORES * HPC * HD, 512 * len(grp)],
                        BF,
                        tag=f"agout{b}_{p}",
                        name=f"agout{b}_{p}",
                        addr_space="Shared",
                    )
                    for p, grp in enumerate(ag_groups)
                ]
                for b in range(B)
            ]

            # ---------------- stage 1: QT/KT/VT projections + rope ----------------
            xt_r = xt.ap().rearrange("(t p) r -> p t r", p=128)
            with nc.named_scope("proj"):
                for r in range(R // 512):
                    rsl = slice(r * 512, (r + 1) * 512)
                    xt_sb = xp.tile([128, KT, 512], BF, tag="xt")
                    if r == 0:
                        for kg in range(4):
                            nc.sync.dma_start(
                                out=xt_sb[:, 4 * kg : 4 * kg + 4, :],
                                in_=xt_r[:, 4 * kg : 4 * kg + 4, rsl],
                            )
                    else:
                        nc.sync.dma_start(out=xt_sb[:], in_=xt_r[:, :, rsl])
                    plan = [(wq_sb, h * HD, qts[h], True) for h in range(HPC)]
                    plan.append((wk_sb, 0, ktile, True))
                    plan.append((wv_sb, 0, vtt, False))
                    for wsb, c0, dst, roped in plan:
                        ps = ps_a.tile([128, 512], F32, tag="ps_a")
                        for k in range(KT):
                            nc.tensor.matmul(
                                ps[:],
                                wsb[:, k, c0 : c0 + HD],
                                xt_sb[:, k, :],
                                start=(k == 0),
                                stop=(k == KT - 1),
                            )
                        if roped:
                            raw = ropep.tile([128, 512], BF, tag="raw")
                            nc.scalar.activation(raw[:], ps[:], AF.Copy)
                            swp = ropep.tile([128, 512], BF, tag="swp")
                            nc.sync.dma_start(out=swp[0:64, :], in_=raw[64:128, :])
                            nc.sync.dma_start(out=swp[64:128, :], in_=raw[0:64, :])
                            t1 = ropep.tile([128, 512], BF, tag="t1")
                            nc.vector.tensor_mul(t1[:], raw[:], cs_sb[:, rsl])
                            t2 = ropep.tile([128, 512], BF, tag="t2")
                            nc.vector.tensor_mul(t2[:], swp[:], sn_sb[:, rsl])
                            nc.vector.tensor_add(dst[:, rsl], t1[:], t2[:])
                        else:
                            nc.scalar.activation(dst[:, rsl], ps[:], AF.Copy)

                # V: [d, row] -> [row, d] via PE transpose
                for rt in range(R // 128):
                    tp = ps_a.tile([128, 128], BF, tag="ps_a")
                    nc.tensor.transpose(tp[:], vtt[:, rt * 128 : (rt + 1) * 128], eye_sb[:])
                    nc.vector.tensor_copy(v3[:, rt, :], tp[:])

            # wo load can happen any time before stage 3
            nc.scalar.dma_start(out=wo_sb[:], in_=wo.ap().rearrange("(t p) c -> p t c", p=128))

            # ---------------- stage 2: attention (+ stage 3 interleaved) ----------------
            def wo_lhs_dma(b, Jq):
                lhs = wop.tile([128, KT, 512], BF, tag="lhs", name="lhs")
                if last_j_split and Jq == nJ - 1:
                    # even feature tiles (h=0 heads) then odd (h=1 heads):
                    # lhs[:, 0:KT//2] <- h0 gather, lhs[:, KT//2:] <- h1 gather
                    for hh in range(HPC):
                        nc.sync.dma_start(
                            out=lhs[:, hh * (KT // 2) : (hh + 1) * (KT // 2), :],
                            in_=agh_out[b][hh].rearrange("(t p) q -> p t q", p=128),
                        )
                else:
                    o = off_of[Jq]
                    nc.sync.dma_start(
                        out=lhs[:],
                        in_=ag_out[b][piece_of[Jq]].rearrange("(t p) q -> p t q", p=128)[
                            :, :, o : o + 512
                        ],
                    )
                return lhs

            def wo_group(b, Jq, lhs=None):
                # out[q, oc] for q-rows [b*S + Jq*512, +512)
                if lhs is None:
                    lhs = wo_lhs_dma(b, Jq)
                if last_j_split and Jq == nJ - 1:
                    # feature tile order: evens (h0 block) then odds (h1 block)
                    ft_src = [(f // 2, f) for f in range(0, KT, 2)] + [
                        (KT // 2 + f // 2, f) for f in range(1, KT, 2)
                    ]
                else:
                    ft_src = [(f, f) for f in range(KT)]
                for sub in range(4):
                    ops = ps_wo.tile([128, OC], F32, tag="ps_wo", name="ops")
                    for idx, (slot, ft) in enumerate(ft_src):
                        nc.tensor.matmul(
                            ops[:],
                            lhs[:, slot, sub * 128 : (sub + 1) * 128],
                            wo_sb[:, ft, :],
                            start=(idx == 0),
                            stop=(idx == KT - 1),
                        )
                    ob = outp.tile([128, OC], F32, tag="ob", name="ob")
                    nc.vector.tensor_copy(ob[:], ops[:])
                    r0 = b * S + Jq * 512 + sub * 128
                    nc.sync.dma_start(out=out[r0 : r0 + 128, :], in_=ob[:])

            for b in range(B):
                with nc.named_scope(f"attn{b}"):
                    # Continuous software pipeline across all chunks of this
                    # batch: scoresT/exp run PIPE_DEPTH steps ahead of the
                    # ones/PV consumers so the PE never drains at chunk
                    # boundaries. Diagonal (masked) k-tiles are emitted first
                    # within each chunk so their mask-multiply has slack.
                    PIPE_DEPTH = 8
                    pend = []

                    def flush_one(e):
                        (ctx, idx, i, q0, ex) = e
                        sums, pv, ktiles, h, J = ctx
                        nc.tensor.matmul(
                            pv[:, q0:512],
                            v3[:, b * (S // 128) + i, :],
                            ex[:, q0:512],
                            start=(idx == 0),
                            stop=(idx == ktiles - 1),
                        )
                        nc.tensor.matmul(
                            sums[:, q0:512],
                            ones128[:],
                            ex[:, q0:512],
                            start=(idx == 0),
                            stop=(idx == ktiles - 1),
                        )
                        if idx == ktiles - 1:
                            finish_chunk(ctx)

                    def finish_chunk(ctx):
                        sums, pv, ktiles, h, J = ctx
                        recf = smallp.tile([1, 512], F32, tag="recf", name="recf")
                        nc.vector.reciprocal_approx_fast(recf[:], sums[:])
                        rec = smallp.tile([1, 512], BF, tag="rec", name="rec")
                        with nc.allow_low_precision(
                            reason="per-column softmax denominators; bf16 ok"
                        ):
                            nc.vector.tensor_copy(rec[:], recf[:])
                        bc = ps_a.tile([128, 512], F32, tag="ps_a", name="bc")
                        nc.tensor.matmul(bc[:], ones1[:], rec[:], start=True, stop=True)
                        un = outp.tile([128, 512], BF, tag="un", name="un")
                        nc.vector.tensor_copy(un[:], pv[:])
                        at = outp.tile([128, 512], BF, tag="at", name="at")
                        nc.vector.tensor_mul(at[:], un[:], bc[:])
                        if last_j_split and J == nJ - 1:
                            nc.sync.dma_start(out=agh_in[b][h][:, :], in_=at[:])
                            nc.gpsimd.collective_compute(
                                "AllGather",
                                mybir.AluOpType.bypass,
                                replica_groups=[list(range(N_CORES))],
                                ins=[agh_in[b][h].opt()],
                                outs=[agh_out[b][h].opt()],
                            )
                        else:
                            p = piece_of[J]
                            o = off_of[J]
                            nc.sync.dma_start(
                                out=ag_in[b][p][h * 128 : (h + 1) * 128, o : o + 512],
                                in_=at[:],
                            )
                            if h == HPC - 1 and J == ag_groups[p][-1]:
                                nc.gpsimd.collective_compute(
                                    "AllGather",
                                    mybir.AluOpType.bypass,
                                    replica_groups=[list(range(N_CORES))],
                                    ins=[ag_in[b][p].opt()],
                                    outs=[ag_out[b][p].opt()],
                                )
                        if h == HPC - 1 and b == 1:
                            wo_group(0, J)
                            if J == nJ - 1 and nJ >= 4:
                                # batch-1 piece 0 (q-chunks 0-1) gathered
                                # long ago; start its wo early
                                wo_group(1, 0)

                    for J in range(nJ):
                        for h in range(HPC):
                            qt_h = qts[h]
                            ktiles = 4 * J + 4
                            sums = ps_sum.tile([1, 512], F32, tag="sums", name="sums")
                            pv = ps_pv.tile([128, 512], F32, tag="pv", name="pv")
                            ctx = (sums, pv, ktiles, h, J)
                            order = list(range(4 * J, 4 * J + 4)) + list(range(0, 4 * J))
                            for idx, i in enumerate(order):
                                m = i - 4 * J
                                q0 = m * 128 if m >= 0 else 0
                                sps = ps_a.tile([128, 512], F32, tag="ps_a", name="sps")
                                nc.tensor.matmul(
                                    sps[:, q0:512],
                                    ktile[:, b * S + i * 128 : b * S + (i + 1) * 128],
                                    qt_h[:, b * S + J * 512 + q0 : b * S + (J + 1) * 512],
                                    start=True,
                                    stop=True,
                                )
                                ex = expp.tile([128, 512], BF, tag="ex", name="ex")
                                nc.scalar.activation(
                                    ex[:, q0:512], sps[:, q0:512], AF.Exp, scale=SCALE
                                )
                                if m >= 0:
                                    nc.vector.tensor_mul(
                                        ex[:, q0 : q0 + 128],
                                        ex[:, q0 : q0 + 128],
                                        bm_sb[:],
                                    )
                                pend.append((ctx, idx, i, q0, ex))
                                if len(pend) > PIPE_DEPTH:
                                    flush_one(pend.pop(0))
                    while pend:
                        flush_one(pend.pop(0))

            # ---------------- stage 3 tail: wo for batch 1 ----------------
            with nc.named_scope("wo1"):
                for Jq in range((1 if nJ >= 4 else 0), nJ):
                    wo_group(1, Jq)

    nc.compile()
    return nc


def prepare_in_maps(x, freqs_cos, freqs_sin, wq, wk, wv, wo, S=2048):
    """Host-side sharding / layout prep. Pure dtype casts and index shuffles."""
    perm = np.concatenate([np.arange(0, HD, 2), np.arange(1, HD, 2)])

    xt = np.ascontiguousarray(x.reshape(B * S, DIM).T).astype(BF16NP)
    wq_p = wq.reshape(DIM, N_HEADS, HD)[:, :, perm]
    wk_p = wk.reshape(DIM, N_KV, HD)[:, :, perm]

    cs_blk = np.tile(freqs_cos.T, (2, 1))  # [128, S]
    sn_blk = np.concatenate([-freqs_sin.T, freqs_sin.T], axis=0)
    cs_full = np.tile(cs_blk, (1, B)).astype(BF16NP)
    sn_full = np.tile(sn_blk, (1, B)).astype(BF16NP)

    ii = np.arange(128)
    binmask = (ii[:, None] <= ii[None, :]).astype(np.float32).astype(BF16NP)
    eye = np.eye(128, dtype=np.float32).astype(BF16NP)

    in_maps = []
    for c in range(N_CORES):
        in_maps.append(
            dict(
                xt=xt,
                wq=np.ascontiguousarray(
                    wq_p[:, HPC * c : HPC * (c + 1), :].reshape(DIM, HPC * HD)
                ).astype(BF16NP),
                wk=np.ascontiguousarray(wk_p[:, c, :]).astype(BF16NP),
                wv=np.ascontiguousarray(wv[:, c * HD : (c + 1) * HD]).astype(BF16NP),
                wo=np.ascontiguousarray(wo[:, c * OC : (c + 1) * OC]).astype(BF16NP),
                cs=cs_full,
                sn=sn_full,
                bm=binmask,
                eye=eye,
            )
        )
    return in_maps


_NC_CACHE = {}


def _get_nc(S=2048):
    if S not in _NC_CACHE:
        _NC_CACHE[S] = build_kernel(S)
    return _NC_CACHE[S]


def kernel(x, freqs_cos, freqs_sin, wq, wk, wv, wo):
    S = x.shape[1]
    nc = _get_nc(S)
    in_maps = prepare_in_maps(x, freqs_cos, freqs_sin, wq, wk, wv, wo, S=S)
    res = run_bass_kernel_spmd(nc, in_maps, core_ids=list(range(N_CORES)))
    parts = [res.results[c]["out"] for c in range(N_CORES)]
    full = np.concatenate(parts, axis=1).astype(np.float32)
    return full.reshape(B, S, DIM)


# revision 17
# speedup vs baseline: 1.0501x; 1.0501x over previous
"""GQA causal attention (B=2, S=2048, D=2048, 16 Q heads / 8 KV heads) on 8 TRN2
NeuronCores, tensor-parallel over heads: core c owns Q heads {2c, 2c+1} and KV
head c, plus the matching output-column shard of wo.

Layout strategy (everything lands in its natural matmul layout, no transposes
of activations except V):
  - host: x is transposed to xT [D, B*S] and cast to bf16; wq/wk columns are
    de-interleaved per head (RoPE pairs (2i,2i+1) -> rows (i, i+64)) so RoPE
    becomes a partition-block swap + elementwise muls.
  - stage 1: QT/KT/VT [128, B*S] = w.T @ x.T accumulated over 16 k-tiles,
    RoPE applied to QT/KT in [d, row] layout via an SBUF->SBUF partition swap
    DMA and two DVE multiplies with host-built [c;c] / [-s;s] tables.
    VT is PE-transposed to V [row, d] (needed as PV stationary operand).
  - stage 2 (per head, batch, 512-wide q-chunk): scoresT [k,q] = KT-tile.T @ QT
    (causal: only k-tiles at or below the diagonal, diagonal k-tiles start at
    column offset so nothing above the diagonal is computed; the in-block
    triangle is zeroed with a 0/1 bf16 mask multiply on the exp output),
    exp on ACT with fused 1/sqrt(dh) scale (ACT runs exp only in stage 2 so
    its activation table never reloads), softmax denominators via ones-vector
    matmuls (partition_sum trick), PV = V-tile.T @ expT accumulated in PSUM,
    normalization by broadcasting reciprocal sums through a rank-1 matmul and
    one DVE multiply.
  - per batch: AllGather of the normalized attnT [256, S] shards over the 8
    cores (overlaps the next batch's attention / wo compute).
  - stage 3: out[q, oc] = attnT_full-tile.T @ wo-shard accumulated over the
    16 feature tiles; written as [B*S, 256] f32, host concatenates shards.
"""

import numpy as np

try:
    import concourse.bass as bass
except ImportError:  # pragma: no cover - container staging path
    import sys

    sys.path.insert(0, "/opt/trn_rl_repo")
    import concourse.bass as bass

import concourse.bacc as bacc
import concourse.mybir as mybir
import concourse.tile as tile
from concourse.bass_utils import run_bass_kernel_spmd
import ml_dtypes

BF16NP = ml_dtypes.bfloat16
N_CORES = 8
B = 2
DIM = 2048
HD = 128
N_HEADS = 16
N_KV = 8
HPC = N_HEADS // N_CORES  # q heads per core (2)
OC = DIM // N_CORES  # wo output columns per core (256)
KT = DIM // 128  # contraction tiles (16)

F32 = mybir.dt.float32
BF = mybir.dt.bfloat16
AF = mybir.ActivationFunctionType

SCALE = float(1.0 / np.sqrt(HD))


def build_kernel(S=2048):
    R = B * S
    nc = bacc.Bacc(None, target_bir_lowering=False, debug=False, num_devices=N_CORES)

    xt = nc.declare_dram_parameter("xt", [DIM, R], BF, isOutput=False)
    wq = nc.declare_dram_parameter("wq", [DIM, HPC * HD], BF, isOutput=False)
    wk = nc.declare_dram_parameter("wk", [DIM, HD], BF, isOutput=False)
    wv = nc.declare_dram_parameter("wv", [DIM, HD], BF, isOutput=False)
    wo = nc.declare_dram_parameter("wo", [DIM, OC], BF, isOutput=False)
    cs = nc.declare_dram_parameter("cs", [HD, R], BF, isOutput=False)
    sn = nc.declare_dram_parameter("sn", [HD, R], BF, isOutput=False)
    bm = nc.declare_dram_parameter("bm", [128, 128], BF, isOutput=False)
    eye = nc.declare_dram_parameter("eye", [128, 128], BF, isOutput=False)
    out = nc.declare_dram_parameter("out", [R, OC], F32, isOutput=True)

    with tile.TileContext(nc) as tc:
        with (
            tc.tile_pool(name="const", bufs=1) as constp,
            tc.tile_pool(name="acts", bufs=1) as actp,
            tc.tile_pool(name="x", bufs=2) as xp,
            tc.tile_pool(name="rope", bufs=3) as ropep,
            tc.tile_pool(name="exps", bufs=9) as expp,
            tc.tile_pool(name="small", bufs=3) as smallp,
            tc.tile_pool(name="outs", bufs=4) as outp,
            tc.tile_pool(name="wos", bufs=3) as wop,
            tc.tile_pool(name="ps_a", bufs=4, space="PSUM") as ps_a,
            tc.tile_pool(name="ps_wo", bufs=1, space="PSUM") as ps_wo,
            tc.tile_pool(name="ps_pv", bufs=2, space="PSUM") as ps_pv,
            tc.tile_pool(name="ps_sum", bufs=1, space="PSUM") as ps_sum,
            tc.tile_pool(name="dram", bufs=1, space="DRAM") as dramp,
        ):
            # ---------------- constants ----------------
            # Load order matters for startup latency: the first projection
            # matmuls need wq + the first xt chunk; wo is only needed in
            # stage 3 (emitted last). Weight loads go on the scalar-engine
            # HWDGE queue so they don't serialize behind the xt stream on
            # sync.
            wq_sb = constp.tile([128, KT, HPC * HD], BF, tag="wq")
            nc.scalar.dma_start(out=wq_sb[:], in_=wq.ap().rearrange("(t p) c -> p t c", p=128))
            wk_sb = constp.tile([128, KT, HD], BF, tag="wk")
            nc.scalar.dma_start(out=wk_sb[:], in_=wk.ap().rearrange("(t p) c -> p t c", p=128))
            wv_sb = constp.tile([128, KT, HD], BF, tag="wv")
            nc.scalar.dma_start(out=wv_sb[:], in_=wv.ap().rearrange("(t p) c -> p t c", p=128))
            cs_sb = constp.tile([128, R], BF, tag="cs")
            nc.scalar.dma_start(out=cs_sb[:], in_=cs.ap())
            sn_sb = constp.tile([128, R], BF, tag="sn")
            nc.scalar.dma_start(out=sn_sb[:], in_=sn.ap())
            bm_sb = constp.tile([128, 128], BF, tag="bm")
            nc.scalar.dma_start(out=bm_sb[:], in_=bm.ap())
            eye_sb = constp.tile([128, 128], BF, tag="eye")
            nc.scalar.dma_start(out=eye_sb[:], in_=eye.ap())
            ones128 = constp.tile([128, 1], BF, tag="ones128")
            nc.vector.memset(ones128[:], 1.0)
            ones1 = constp.tile([1, 128], BF, tag="ones1")
            nc.vector.memset(ones1[:], 1.0)
            wo_sb = constp.tile([128, KT, OC], BF, tag="wo")

            # per-head-per-batch activations (resident)
            qts = [actp.tile([128, R], BF, tag=f"qt{h}", name=f"qt{h}") for h in range(HPC)]
            ktile = actp.tile([128, R], BF, tag="kt")
            vtt = actp.tile([128, R], BF, tag="vt")
            v3 = actp.tile([128, R // 128, 128], BF, tag="v3")

            nJ = S // 512
            last_j_split = False  # per-head final gather measured slower
            if nJ >= 4:
                ag_groups = [[0, 1]] + [[j] for j in range(2, nJ)]
            else:
                ag_groups = [[j] for j in range(nJ)]
            piece_of = {}
            off_of = {}
            for p, grp in enumerate(ag_groups):
                for gi, j in enumerate(grp):
                    piece_of[j] = p
                    off_of[j] = gi * 512
            ag_in = [
                [
                    dramp.tile(
                        [HPC * HD, 512 * len(grp)],
                        BF,
                        tag=f"agin{b}_{p}",
                        name=f"agin{b}_{p}",
                    )
                    for p, grp in enumerate(ag_groups)
                ]
                for b in range(B)
            ]
            agh_in = [
                [
                    dramp.tile([HD, 512], BF, tag=f"aghin{b}_{h}", name=f"aghin{b}_{h}")
                    for h in range(HPC)
                ]
                for b in range(B)
            ]
            agh_out = [
                [
                    dramp.tile(
                        [N_CORES * HD, 512],
                        BF,
                        tag=f"aghout{b}_{h}",
                        name=f"aghout{b}_{h}",
                        addr_space="Shared",
                    )
                    for h in range(HPC)
                ]
                for b in range(B)
            ]
            ag_out = [
                [
                    dramp.tile(
                        [N_CORES * HPC * HD, 512 * len(grp)],
                        BF,
                        tag=f"agout{b}_{p}",
                        name=f"agout{b}_{p}",
                        addr_space="Shared",
                    )
                    for p, grp in enumerate(ag_groups)
                ]
                for b in range(B)
            ]

            # ---------------- stage 1: QT/KT/VT projections + rope ----------------
            xt_r = xt.ap().rearrange("(t p) r -> p t r", p=128)
            with nc.named_scope("proj"):
                for r in range(R // 512):
                    rsl = slice(r * 512, (r + 1) * 512)
                    xt_sb = xp.tile([128, KT, 512], BF, tag="xt")
                    if r == 0:
                        for kg in range(4):
                            nc.sync.dma_start(
                                out=xt_sb[:, 4 * kg : 4 * kg + 4, :],
                                in_=xt_r[:, 4 * kg : 4 * kg + 4, rsl],
                            )
                    else:
                        nc.sync.dma_start(out=xt_sb[:], in_=xt_r[:, :, rsl])
                    plan = [(wq_sb, h * HD, qts[h], True) for h in range(HPC)]
                    plan.append((wk_sb, 0, ktile, True))
                    plan.append((wv_sb, 0, vtt, False))
                    for wsb, c0, dst, roped in plan:
                        ps = ps_a.tile([128, 512], F32, tag="ps_a")
                        for k in range(KT):
                            nc.tensor.matmul(
                                ps[:],
                                wsb[:, k, c0 : c0 + HD],
                                xt_sb[:, k, :],
                                start=(k == 0),
                                stop=(k == KT - 1),
                            )
                        if roped:
                            raw = ropep.tile([128, 512], BF, tag="raw")
                            nc.scalar.activation(raw[:], ps[:], AF.Copy)
                            swp = ropep.tile([128, 512], BF, tag="swp")
                            nc.sync.dma_start(out=swp[0:64, :], in_=raw[64:128, :])
                            nc.sync.dma_start(out=swp[64:128, :], in_=raw[0:64, :])
                            t1 = ropep.tile([128, 512], BF, tag="t1")
                            nc.vector.tensor_mul(t1[:], raw[:], cs_sb[:, rsl])
                            t2 = ropep.tile([128, 512], BF, tag="t2")
                            nc.vector.tensor_mul(t2[:], swp[:], sn_sb[:, rsl])
                            nc.vector.tensor_add(dst[:, rsl], t1[:], t2[:])
                        else:
                            nc.scalar.activation(dst[:, rsl], ps[:], AF.Copy)

                # V: [d, row] -> [row, d] via PE transpose
                for rt in range(R // 128):
                    tp = ps_a.tile([128, 128], BF, tag="ps_a")
                    nc.tensor.transpose(tp[:], vtt[:, rt * 128 : (rt + 1) * 128], eye_sb[:])
                    nc.vector.tensor_copy(v3[:, rt, :], tp[:])

            # wo load can happen any time before stage 3
            nc.scalar.dma_start(out=wo_sb[:], in_=wo.ap().rearrange("(t p) c -> p t c", p=128))

            # ---------------- stage 2: attention (+ stage 3 interleaved) ----------------
            def wo_lhs_dma(b, Jq):
                lhs = wop.tile([128, KT, 512], BF, tag="lhs", name="lhs")
                if last_j_split and Jq == nJ - 1:
                    # even feature tiles (h=0 heads) then odd (h=1 heads):
                    # lhs[:, 0:KT//2] <- h0 gather, lhs[:, KT//2:] <- h1 gather
                    for hh in range(HPC):
                        nc.sync.dma_start(
                            out=lhs[:, hh * (KT // 2) : (hh + 1) * (KT // 2), :],
                            in_=agh_out[b][hh].rearrange("(t p) q -> p t q", p=128),
                        )
                else:
                    o = off_of[Jq]
                    nc.sync.dma_start(
                        out=lhs[:],
                        in_=ag_out[b][piece_of[Jq]].rearrange("(t p) q -> p t q", p=128)[
                            :, :, o : o + 512
                        ],
                    )
                return lhs

            def wo_group(b, Jq, lhs=None):
                # out[q, oc] for q-rows [b*S + Jq*512, +512)
                if lhs is None:
                    lhs = wo_lhs_dma(b, Jq)
                if last_j_split and Jq == nJ - 1:
                    # feature tile order: evens (h0 block) then odds (h1 block)
                    ft_src = [(f // 2, f) for f in range(0, KT, 2)] + [
                        (KT // 2 + f // 2, f) for f in range(1, KT, 2)
                    ]
                else:
                    ft_src = [(f, f) for f in range(KT)]
                for sub in range(4):
                    ops = ps_wo.tile([128, OC], F32, tag="ps_wo", name="ops")
                    for idx, (slot, ft) in enumerate(ft_src):
                        nc.tensor.matmul(
                            ops[:],
                            lhs[:, slot, sub * 128 : (sub + 1) * 128],
                            wo_sb[:, ft, :],
                            start=(idx == 0),
                            stop=(idx == KT - 1),
                        )
                    ob = outp.tile([128, OC], F32, tag="ob", name="ob")
                    nc.vector.tensor_copy(ob[:], ops[:])
                    r0 = b * S + Jq * 512 + sub * 128
                    nc.sync.dma_start(out=out[r0 : r0 + 128, :], in_=ob[:])

            for b in range(B):
                with nc.named_scope(f"attn{b}"):
                    # Continuous software pipeline across all chunks of this
                    # batch: scoresT/exp run PIPE_DEPTH steps ahead of the
                    # ones/PV consumers so the PE never drains at chunk
                    # boundaries. Diagonal (masked) k-tiles are emitted first
                    # within each chunk so their mask-multiply has slack.
                    PIPE_DEPTH = 6
                    pend = []

                    def flush_one(e):
                        (ctx, idx, i, q0, ex) = e
                        sums, pv, ktiles, h, J = ctx
                        nc.tensor.matmul(
                            sums[:, q0:512],
                            ones128[:],
                            ex[:, q0:512],
                            start=(idx == 0),
                            stop=(idx == ktiles - 1),
                        )
                        nc.tensor.matmul(
                            pv[:, q0:512],
                            v3[:, b * (S // 128) + i, :],
                            ex[:, q0:512],
                            start=(idx == 0),
                            stop=(idx == ktiles - 1),
                        )
                        if idx == ktiles - 1:
                            finish_chunk(ctx)

                    def finish_chunk(ctx):
                        sums, pv, ktiles, h, J = ctx
                        recf = smallp.tile([1, 512], F32, tag="recf", name="recf")
                        nc.vector.reciprocal_approx_fast(recf[:], sums[:])
                        rec = smallp.tile([1, 512], BF, tag="rec", name="rec")
                        with nc.allow_low_precision(
                            reason="per-column softmax denominators; bf16 ok"
                        ):
                            nc.vector.tensor_copy(rec[:], recf[:])
                        bc = ps_a.tile([128, 512], F32, tag="ps_a", name="bc")
                        nc.tensor.matmul(bc[:], ones1[:], rec[:], start=True, stop=True)
                        un = outp.tile([128, 512], BF, tag="un", name="un")
                        nc.vector.tensor_copy(un[:], pv[:])
                        at = outp.tile([128, 512], BF, tag="at", name="at")
                        nc.vector.tensor_mul(at[:], un[:], bc[:])
                        if last_j_split and J == nJ - 1:
                            nc.sync.dma_start(out=agh_in[b][h][:, :], in_=at[:])
                            nc.gpsimd.collective_compute(
                                "AllGather",
                                mybir.AluOpType.bypass,
                                replica_groups=[list(range(N_CORES))],
                                ins=[agh_in[b][h].opt()],
                                outs=[agh_out[b][h].opt()],
                            )
                        else:
                            p = piece_of[J]
                            o = off_of[J]
                            nc.sync.dma_start(
                                out=ag_in[b][p][h * 128 : (h + 1) * 128, o : o + 512],
                                in_=at[:],
                            )
                            if h == HPC - 1 and J == ag_groups[p][-1]:
                                nc.gpsimd.collective_compute(
                                    "AllGather",
                                    mybir.AluOpType.bypass,
                                    replica_groups=[list(range(N_CORES))],
                                    ins=[ag_in[b][p].opt()],
                                    outs=[ag_out[b][p].opt()],
                                )
                        if h == HPC - 1 and b == 1:
                            wo_group(0, J)
                            if J == nJ - 1 and nJ >= 4:
                                # batch-1 piece 0 (q-chunks 0-1) gathered
                                # long ago; start its wo early
                                wo_group(1, 0)

                    for J in range(nJ):
                        for h in range(HPC):
                            qt_h = qts[h]
                            ktiles = 4 * J + 4
                            sums = ps_sum.tile([1, 512], F32, tag="sums", name="sums")
                            pv = ps_pv.tile([128, 512], F32, tag="pv", name="pv")
                            ctx = (sums, pv, ktiles, h, J)
                            order = list(range(4 * J, 4 * J + 4)) + list(range(0, 4 * J))
                            for idx, i in enumerate(order):
                                m = i - 4 * J
                                q0 = m * 128 if m >= 0 else 0
                                sps = ps_a.tile([128, 512], F32, tag="ps_a", name="sps")
                                nc.tensor.matmul(
                                    sps[:, q0:512],
                                    ktile[:, b * S + i * 128 : b * S + (i + 1) * 128],
                                    qt_h[:, b * S + J * 512 + q0 : b * S + (J + 1) * 512],
                                    start=True,
                                    stop=True,
                                )
                                ex = expp.tile([128, 512], BF, tag="ex", name="ex")
                                nc.scalar.activation(
                                    ex[:, q0:512], sps[:, q0:512], AF.Exp, scale=SCALE
                                )
                                if m >= 0:
                                    nc.vector.tensor_mul(
                                        ex[:, q0 : q0 + 128],
                                        ex[:, q0 : q0 + 128],
                                        bm_sb[:],
                                    )
                                pend.append((ctx, idx, i, q0, ex))
                                if len(pend) > PIPE_DEPTH:
                                    flush_one(pend.pop(0))
                    while pend:
                        flush_one(pend.pop(0))

            # ---------------- stage 3 tail: wo for batch 1 ----------------
            with nc.named_scope("wo1"):
                for Jq in range((1 if nJ >= 4 else 0), nJ):
                    wo_group(1, Jq)

    nc.compile()
    return nc


def prepare_in_maps(x, freqs_cos, freqs_sin, wq, wk, wv, wo, S=2048):
    """Host-side sharding / layout prep. Pure dtype casts and index shuffles."""
    perm = np.concatenate([np.arange(0, HD, 2), np.arange(1, HD, 2)])

    xt = np.ascontiguousarray(x.reshape(B * S, DIM).T).astype(BF16NP)
    wq_p = wq.reshape(DIM, N_HEADS, HD)[:, :, perm]
    wk_p = wk.reshape(DIM, N_KV, HD)[:, :, perm]

    cs_blk = np.tile(freqs_cos.T, (2, 1))  # [128, S]
    sn_blk = np.concatenate([-freqs_sin.T, freqs_sin.T], axis=0)
    cs_full = np.tile(cs_blk, (1, B)).astype(BF16NP)
    sn_full = np.tile(sn_blk, (1, B)).astype(BF16NP)

    ii = np.arange(128)
    binmask = (ii[:, None] <= ii[None, :]).astype(np.float32).astype(BF16NP)
    eye = np.eye(128, dtype=np.float32).astype(BF16NP)

    in_maps = []
    for c in range(N_CORES):
        in_maps.append(
            dict(
                xt=xt,
                wq=np.ascontiguousarray(
                    wq_p[:, HPC * c : HPC * (c + 1), :].reshape(DIM, HPC * HD)
                ).astype(BF16NP),
                wk=np.ascontiguousarray(wk_p[:, c, :]).astype(BF16NP),
                wv=np.ascontiguousarray(wv[:, c * HD : (c + 1) * HD]).astype(BF16NP),
                wo=np.ascontiguousarray(wo[:, c * OC : (c + 1) * OC]).astype(BF16NP),
                cs=cs_full,
                sn=sn_full,
                bm=binmask,
                eye=eye,
            )
        )
    return in_maps


_NC_CACHE = {}


def _get_nc(S=2048):
    if S not in _NC_CACHE:
        _NC_CACHE[S] = build_kernel(S)
    return _NC_CACHE[S]


def kernel(x, freqs_cos, freqs_sin, wq, wk, wv, wo):
    x, freqs_cos, freqs_sin, wq, wk, wv, wo = (
        np.asarray(a, dtype=np.float32)
        for a in (x, freqs_cos, freqs_sin, wq, wk, wv, wo)
    )
    S = x.shape[1]
    nc = _get_nc(S)
    in_maps = prepare_in_maps(x, freqs_cos, freqs_sin, wq, wk, wv, wo, S=S)
    res = run_bass_kernel_spmd(nc, in_maps, core_ids=list(range(N_CORES)))
    parts = [res.results[c]["out"] for c in range(N_CORES)]
    full = np.concatenate(parts, axis=1).astype(np.float32)
    return full.reshape(B, S, DIM)
